# revision 3
# baseline (speedup 1.0000x reference)
"""Multi-head attention block (nn_Attention) on 8 Trainium2 NeuronCores.

Reference computation (per batch element):
    qkv = x @ w_qkv.T + b_qkv               # [T=1024, 3D], D=768
    q, k, v per head (H=12, Hd=64)
    attn = softmax(q @ k.T / sqrt(Hd))
    out  = (attn @ v) @ w_proj.T + b_proj   # [T, D]

Sharding: pure data parallelism over the batch (B=8) - one batch element per
NeuronCore, weights replicated, no collectives.

Dtypes: x, w_qkv, w_proj stream in as bf16 (halves HBM traffic); q/k stay
f32r (same PE rate as bf16 for moving dims >= 256, ~30x less logit noise);
v, P=exp(S), O and the projection run bf16. PSUM always fp32. The v-bias is
folded into b_proj on the host (attn rows sum to 1), so v staging is a plain
PSUM->SBUF copy.

Per-core pipeline, chunk-major (c = 512-query chunk):
  1. qkT [1536, T] feature-major (bias via DVE from PSUM) and v token-major
     as [v_h | 1] blocks of 65 columns - the ones column makes the O.T
     matmul emit softmax denominators for free.
  2. Attention per (head pair, chunk): S.T = kT.T @ qT per 128-key tile into
     2-bank PSUM groups; exp on ScalarE (1/8 scale folded, bf16 out),
     software-pipelined one key-group ahead of the O.T accumulation
     O'.T [65, tq] = [v_h | 1].T @ P.T. Normalization: InstReciprocal of
     row 64 read straight from PSUM + GPSIMD partition_broadcast + one DVE
     mul from PSUM, wavefront-split in 256-col halves to shorten the
     PSUM-release chain. (Use nc.vector.reciprocal here, NOT
     reciprocal_approx_fast: the custom uop silently corrupts when reading
     PSUM on HW; the first-class instruction is fine.)
  3. out = OT.T-contract @ wT_proj + b_proj', interleaved: chunk-0 token
     tiles run during the chunk-1 attention pass; only the k=5 closure of
     the last tiles trails the final norm.

Scheduling notes (Tile priority = emission order; producers MUST be emitted
before consumers or you get an untracked race): remaining qkv feature tiles,
the second v half and the w_proj load are due-date-ordered fill-in jobs that
keep the PE busy under the ScalarE exp stream (~101 us busy). PE busy is
~144 us (serial floor: matmul cost = moving columns; K=64 row-pairing via
tile_position measured 3x SLOWER on HW, fp8 fails the accuracy gate, so the
S.T/O.T partition waste is unfixable). DMA: one 3D descriptor per logical
tensor, spread across SP/ACT queues; ~8 MB/core total.
"""
import os
import numpy as np

os.environ.setdefault("JAX_COMPILATION_CACHE_DIR", "/tmp/jax_neff_cache")

import concourse.bass as bass
import concourse.bacc as bacc
import concourse.tile as tile
from concourse import mybir

F32 = mybir.dt.float32
F32R = mybir.dt.float32r
BF16 = mybir.dt.bfloat16

B, T, D = 8, 1024, 768
H, HD = 12, 64
SCALE = HD ** -0.5
N_CORES = 8
TT = T // 128       # 8 token tiles
DT = D // 128       # 6 contraction tiles
TQ = 512            # query chunk (moving dim)
NCH = T // TQ       # 2 query chunks
SG = [(0, 2), (2, 4), (4, 6), (6, 8)]  # key-tile groups (2 PSUM banks each)


def _bcast_ap(ap_1d, parts, n):
    return bass.AP(tensor=ap_1d.tensor, offset=ap_1d.offset,
                   ap=[[0, parts], [1, n]])


def build_nc(reps=1):
    nc = bacc.Bacc(trn_type="TRN2", debug=False, num_devices=N_CORES)
    xt_d = nc.dram_tensor("xT", (D, T), BF16, kind="ExternalInput")
    wqkv_d = nc.dram_tensor("wT_qkv", (D, 3 * D), BF16, kind="ExternalInput")
    bqkv_d = nc.dram_tensor("b_qkv", (3 * D,), F32, kind="ExternalInput")
    wproj_d = nc.dram_tensor("wT_proj", (D, D), BF16, kind="ExternalInput")
    bproj_d = nc.dram_tensor("b_proj", (D,), F32, kind="ExternalInput")
    out_d = nc.dram_tensor("out", (T, D), F32, kind="ExternalOutput")

    with tile.TileContext(nc) as tc:
        for _ in range(reps):
            _body(nc, tc, xt_d, wqkv_d, bqkv_d, wproj_d, bproj_d, out_d)
    nc.compile()
    return nc


def _body(nc, tc, xt_d, wqkv_d, bqkv_d, wproj_d, bproj_d, out_d):
    from contextlib import ExitStack
    with ExitStack() as ctx:
        consts = ctx.enter_context(tc.tile_pool(name="consts", bufs=1))
        qkt_pool = ctx.enter_context(tc.tile_pool(name="qkt", bufs=1))
        v_pool = ctx.enter_context(tc.tile_pool(name="vst", bufs=1))
        ot_pool = ctx.enter_context(tc.tile_pool(name="ot", bufs=1))
        x_pool = ctx.enter_context(tc.tile_pool(name="x", bufs=1))
        wqk_pool = ctx.enter_context(tc.tile_pool(name="wqk", bufs=1))
        mm_ps = ctx.enter_context(tc.tile_pool(name="mmps", bufs=2, space="PSUM"))

        bias_qk = consts.tile([128, 12], F32)
        nc.sync.dma_start(bias_qk[:],
                          bqkv_d[0:1536].rearrange("(t p) -> p t", p=128))
        bias_p = consts.tile([128, D], F32)
        ones12 = consts.tile([128, H, 1], BF16)
        nc.vector.memset(ones12[:], 1.0)

        qkT = [qkt_pool.tile([128, T], F32R, name=f"qkT{fi}") for fi in range(12)]
        vst = [v_pool.tile([128, H, 65], BF16, name=f"vst{ti}") for ti in range(TT)]
        OT = [ot_pool.tile([128, T], BF16, name=f"OT{k}") for k in range(DT)]

        xt_r = xt_d.rearrange("(dt p) t -> p dt t", p=128)
        wq_r = wqkv_d.rearrange("(dt p) f -> p dt f", p=128)
        wp_r = wproj_d.rearrange("(dt p) f -> p dt f", p=128)
        xTM = x_pool.tile([128, DT, T], BF16, name="xTM")
        xT = [xTM[:, k, :] for k in range(DT)]
        wTqkM = wqk_pool.tile([128, DT, 1536], BF16, name="wTqkM")
        wTqk = [wTqkM[:, k, :] for k in range(DT)]
        for k in range(DT):
            nc.scalar.dma_start(
                xTM[:, k, 0:512], xt_r[:, k, 0:512])

        def load_x_c1():
            for k in range(DT):
                nc.sync.dma_start(
                    xTM[:, k, 512:1024], xt_r[:, k, 512:1024])

        def emit_fi_chunk(fi, c, with_dma):
            """One qkT feature tile, one 512-token chunk; optionally stream
            the wT_qkv column slice first (one 3D descriptor)."""
            if with_dma:
                if fi in (0, 6):
                    nc.sync.dma_start(
                        wTqkM[:, 0:1, 128 * fi:128 * (fi + 1)],
                        wq_r[:, 0:1, 128 * fi:128 * (fi + 1)])
                    nc.sync.dma_start(
                        wTqkM[:, 1:DT, 128 * fi:128 * (fi + 1)],
                        wq_r[:, 1:DT, 128 * fi:128 * (fi + 1)])
                else:
                    nc.sync.dma_start(
                        wTqkM[:, :, 128 * fi:128 * (fi + 1)],
                        wq_r[:, :, 128 * fi:128 * (fi + 1)])
            pq = mm_ps.tile([128, TQ], F32, tag="mm", name=f"pq{fi}_{c}")
            for k in range(DT):
                nc.tensor.matmul(
                    pq[:], wTqk[k][:, 128 * fi:128 * (fi + 1)],
                    xT[k][:, TQ * c:TQ * (c + 1)],
                    start=(k == 0), stop=(k == DT - 1))
            nc.vector.tensor_scalar_add(
                qkT[fi][:, TQ * c:TQ * (c + 1)], pq[:], bias_qk[:, fi:fi + 1])

        def emit_fi(fi):
            for c in range(NCH):
                emit_fi_chunk(fi, c, with_dma=(c == 0))

        def emit_v_half(c2, wTv, ti_range=None):
            """v columns [384*c2, 384*(c2+1)) for token tiles (heads 6c2..6c2+6)."""
            for ti in (ti_range if ti_range is not None else range(TT)):
                pv = mm_ps.tile([128, 384], F32, tag="mm", name=f"pv{ti}_{c2}")
                for k in range(DT):
                    nc.tensor.matmul(
                        pv[:], xT[k][:, 128 * ti:128 * (ti + 1)],
                        wTv[k][:], start=(k == 0), stop=(k == DT - 1))
                nc.vector.tensor_copy(
                    vst[ti][:, 6 * c2:6 * (c2 + 1), 0:64],
                    pv[:].rearrange("p (h d) -> p h d", d=64))
                nc.gpsimd.tensor_copy(
                    vst[ti][:, 6 * c2:6 * (c2 + 1), 64:65], ones12[:, 0:6, :])

        wshare = ctx.enter_context(tc.tile_pool(name="wshare", bufs=2))

        def load_wv(c2, eng=None):
            eng = eng or nc.sync
            m = wshare.tile([128, DT, 384], BF16, tag="ws", name=f"wTvM{c2}")
            eng.dma_start(
                m[:, :, :],
                wq_r[:, :, 1536 + 384 * c2:1536 + 384 * (c2 + 1)])
            return [m[:, k, :] for k in range(DT)]

        pt_pool = ctx.enter_context(tc.tile_pool(name="pt", bufs=4))
        sums_pool = ctx.enter_context(tc.tile_pool(name="sums", bufs=1))
        rsb_pool = ctx.enter_context(tc.tile_pool(name="rsb", bufs=2))
        s_ps = ctx.enter_context(tc.tile_pool(name="sps", bufs=2, space="PSUM"))
        o_ps = ctx.enter_context(tc.tile_pool(name="ops", bufs=1, space="PSUM"))

        def emit_sg(hp, c, g):
            """S.T matmuls + exp for one key-tile group of a head pair."""
            g0, g1 = SG[g]
            gl = g1 - g0
            sp = [s_ps.tile([128, 512 * gl], F32, tag="s",
                            name=f"sps{c}_{hp}_{g}_{p}") for p in (0, 1)]
            for p in (0, 1):
                qb = 64 * p
                for tkt in range(g0, g1):
                    nc.tensor.matmul(
                        sp[p][:, 512 * (tkt - g0):512 * (tkt - g0 + 1)],
                        qkT[6 + hp][qb:qb + 64, 128 * tkt:128 * (tkt + 1)],
                        qkT[hp][qb:qb + 64, TQ * c:TQ * (c + 1)],
                        start=True, stop=True)
            pt = [pt_pool.tile([128, 512 * gl], BF16, tag=f"pt{p}",
                               name=f"PT{c}_{hp}_{g}_{p}") for p in (0, 1)]
            for p in (0, 1):
                nc.scalar.activation(
                    pt[p][:], sp[p][:], mybir.ActivationFunctionType.Exp,
                    bias=0.0, scale=float(SCALE))
            return pt

        def emit_og(hp, c, po, g, pt):
            g0, g1 = SG[g]
            for p in (0, 1):
                h = 2 * hp + p
                for tk in range(g0, g1):
                    nc.tensor.matmul(
                        po[p][0:65, :], vst[tk][:, h, :],
                        pt[p][:, 512 * (tk - g0):512 * (tk - g0 + 1)],
                        start=(g == 0 and tk == g0),
                        stop=(g == len(SG) - 1 and tk == g1 - 1),
                        skip_group_check=True)

        def emit_norm(hp, c, po):
            # wavefront over 256-col halves: the copy/recip/bcast/mul hops of
            # half 0 overlap half 1, halving the po-release latency.
            # (reciprocal_approx_fast cannot read PSUM on HW, hence the copy.)
            for p in (0, 1):
                sst = sums_pool.tile([128, TQ], F32, tag=f"sums{p}",
                                     name=f"sst{c}_{hp}_{p}")
                rsb = rsb_pool.tile([64, TQ], F32, tag=f"rsb{p}",
                                    name=f"rsb{c}_{hp}_{p}")
                for h in (0, 1):
                    hs = slice(256 * h, 256 * (h + 1))
                    nc.vector.reciprocal(sst[0:1, hs], po[p][64:65, hs])
                    nc.gpsimd.partition_broadcast(rsb[:, hs], sst[0:1, hs])
                    nc.vector.tensor_mul(
                        OT[hp][64 * p:64 * (p + 1), TQ * c + 256 * h:
                               TQ * c + 256 * (h + 1)],
                        po[p][0:64, hs], rsb[:, hs])

        def emit_attn(hp, c, po, pre_pt=(), tail_jobs=()):
            # software-pipelined: S/exp runs up to two groups ahead of the
            # O.T drain (sp psum rotation allows exactly two in flight).
            pts = list(pre_pt)
            for g in range(len(SG)):
                if g >= len(pts):
                    pts.append(emit_sg(hp, c, g))
                if g + 1 < len(SG) and g + 1 >= len(pts):
                    pts.append(emit_sg(hp, c, g + 1))
                emit_og(hp, c, po, g, pts[g])
            for job in tail_jobs:
                job()
            emit_norm(hp, c, po)

        # lead-in: first head pair's q/k, then its first S/exp groups so
        # ScalarE ramps while the PE grinds the v projection.
        emit_fi_chunk(0, 0, with_dma=True)
        emit_fi_chunk(6, 0, with_dma=True)
        load_x_c1()
        po_pre = [o_ps.tile([128, TQ], F32, tag=f"o{p}", name=f"ops0_0_{p}")
                  for p in (0, 1)]
        pre_pt = [emit_sg(0, 0, 0), emit_sg(0, 0, 1)]
        emit_fi_chunk(0, 1, with_dma=False)
        emit_fi_chunk(6, 1, with_dma=False)
        nc.sync.dma_start(bias_p[:], _bcast_ap(bproj_d[0:D], 128, D))
        wTv0 = load_wv(0, eng=nc.scalar)
        # interleave the v projection with head pair 0 / chunk 0 attention so
        # ScalarE's exp stream starts immediately; late S/exp groups are
        # emitted as early as their PSUM/PT slots can possibly free up
        emit_v_half(0, wTv0, ti_range=range(0, 2))
        pre_pt.append(emit_sg(0, 0, 2))
        emit_og(0, 0, po_pre, 0, pre_pt[0])
        emit_v_half(0, wTv0, ti_range=range(2, 4))
        pre_pt.append(emit_sg(0, 0, 3))
        emit_og(0, 0, po_pre, 1, pre_pt[1])
        emit_v_half(0, wTv0, ti_range=range(4, 6))
        emit_og(0, 0, po_pre, 2, pre_pt[2])
        emit_v_half(0, wTv0, ti_range=range(6, 8))
        emit_og(0, 0, po_pre, 3, pre_pt[3])
        emit_norm(0, 0, po_pre)

        # deferred jobs, spread across attention iterations (due-date ordered)
        wTv1 = load_wv(1)
        wTp = {}

        def load_wp():
            for c2 in range(2):
                m = wshare.tile([128, DT, 384], BF16, tag="wsp", name=f"wTpM{c2}")
                nc.sync.dma_start(
                    m[:, :, :], wp_r[:, :, 384 * c2:384 * (c2 + 1)])
                for k in range(DT):
                    wTp[(c2, k)] = m[:, k, :]

        outst = ctx.enter_context(tc.tile_pool(name="outst", bufs=3))

        def emit_proj_partial(ti):
            pps = []
            for c2 in range(2):
                pp = mm_ps.tile([128, 384], F32, tag="mm", name=f"pp{ti}_{c2}")
                for k in range(DT - 1):
                    nc.tensor.matmul(
                        pp[:], OT[k][:, 128 * ti:128 * (ti + 1)],
                        wTp[(c2, k)][:], start=(k == 0), stop=False)
                pps.append(pp)
            return pps

        def emit_proj_finish(ti, pps):
            ob = outst.tile([128, D], F32, tag="ob", name=f"ob{ti}")
            k = DT - 1
            for c2 in range(2):
                nc.tensor.matmul(
                    pps[c2][:], OT[k][:, 128 * ti:128 * (ti + 1)],
                    wTp[(c2, k)][:], start=False, stop=True)
                nc.vector.tensor_add(
                    ob[:, 384 * c2:384 * (c2 + 1)], pps[c2][:],
                    bias_p[:, 384 * c2:384 * (c2 + 1)])
                nc.sync.dma_start(
                    out_d[128 * ti:128 * (ti + 1), 384 * c2:384 * (c2 + 1)],
                    ob[:, 384 * c2:384 * (c2 + 1)])

        def emit_proj(ti):
            ob = outst.tile([128, D], F32, tag="ob", name=f"ob{ti}")
            for c2 in range(2):
                pp = mm_ps.tile([128, 384], F32, tag="mm", name=f"pp{ti}_{c2}")
                for k in range(DT):
                    nc.tensor.matmul(
                        pp[:], OT[k][:, 128 * ti:128 * (ti + 1)],
                        wTp[(c2, k)][:],
                        start=(k == 0), stop=(k == DT - 1))
                nc.vector.tensor_add(
                    ob[:, 384 * c2:384 * (c2 + 1)], pp[:],
                    bias_p[:, 384 * c2:384 * (c2 + 1)])
                nc.sync.dma_start(
                    out_d[128 * ti:128 * (ti + 1), 384 * c2:384 * (c2 + 1)],
                    ob[:, 384 * c2:384 * (c2 + 1)])

        # fill-in jobs between attention iterations, due-date ordered.
        # c=0 pass: remaining qkv feature tiles, v half 2, w_proj load.
        # c=1 pass: chunk-0 token tiles of the output projection.
        jobs = {
            (0, 0): [lambda: emit_fi(1), lambda: emit_fi(7)],
            (0, 1): [lambda: emit_fi(2), lambda: emit_fi(8),
                     lambda: emit_v_half(1, wTv1, ti_range=range(0, 4))],
            (0, 2): [lambda: emit_v_half(1, wTv1, ti_range=range(4, TT)),
                     lambda: emit_fi(3), lambda: emit_fi(9)],
            (0, 3): [lambda: emit_fi(4), lambda: emit_fi(10), lambda: load_wp()],
            (0, 4): [lambda: emit_fi(5), lambda: emit_fi(11)],
            (1, 1): [lambda: emit_proj(0)],
            (1, 2): [lambda: emit_proj(1)],
            (1, 3): [lambda: emit_proj(2)],
            (1, 4): [lambda: emit_proj(3)],
        }

        held = {}
        for c in range(NCH):
            for hp in range(6):
                if not (hp == 0 and c == 0):
                    po = [o_ps.tile([128, TQ], F32, tag=f"o{p}",
                                    name=f"ops{c}_{hp}_{p}") for p in (0, 1)]
                    tail = ()
                    if (c, hp) == (1, 5):
                        tail = (lambda: held.__setitem__(4, emit_proj_partial(4)),)
                    emit_attn(hp, c, po, tail_jobs=tail)
                for job in jobs.get((c, hp), []):
                    job()
        # remaining projection token tiles (chunk-1 tokens)
        emit_proj_finish(4, held[4])
        for ti in range(5, TT):
            emit_proj(ti)


_CACHE = {}


def _get_runner():
    if "runner" in _CACHE:
        return _CACHE["runner"]
    import jax
    from jax.sharding import Mesh, PartitionSpec
    from jax.experimental.shard_map import shard_map
    from concourse import bass2jax
    from concourse.bass2jax import _bass_exec_p, partition_id_tensor

    nc = build_nc()
    bass2jax.install_neuronx_cc_hook()
    partition_name = nc.partition_id_tensor.name if nc.partition_id_tensor else None
    in_names, out_names, out_avals = [], [], []
    for alloc in nc.m.functions[0].allocations:
        if not isinstance(alloc, mybir.MemoryLocationSet):
            continue
        name = alloc.memorylocations[0].name
        if alloc.kind == "ExternalInput":
            if name != partition_name:
                in_names.append(name)
        elif alloc.kind == "ExternalOutput":
            out_names.append(name)
            out_avals.append(jax.core.ShapedArray(
                tuple(alloc.tensor_shape), mybir.dt.np(alloc.dtype)))
    all_in = list(in_names) + list(out_names)
    if partition_name is not None:
        all_in.append(partition_name)

    def _jbody(*args):
        operands = list(args)
        if partition_name is not None:
            operands.append(partition_id_tensor())
        return tuple(_bass_exec_p.bind(
            *operands, out_avals=tuple(out_avals), in_names=tuple(all_in),
            out_names=tuple(out_names), lowering_input_output_aliases=(),
            sim_require_finite=True, sim_require_nnan=True, nc=nc))

    devices = jax.devices()[:N_CORES]
    mesh = Mesh(np.asarray(devices), ("core",))
    # xT is batch-sharded on the core axis; weights/biases are replicated.
    sharded_in = {"xT"}
    in_specs = tuple(
        PartitionSpec("core") if n in sharded_in else PartitionSpec()
        for n in in_names
    ) + (PartitionSpec("core"),) * len(out_names)
    fn = jax.jit(
        shard_map(_jbody, mesh=mesh, in_specs=in_specs,
                  out_specs=(PartitionSpec("core"),) * len(out_names),
                  check_rep=False),
        keep_unused=True)
    _CACHE["runner"] = (fn, in_names, out_names, out_avals, mesh)
    return _CACHE["runner"]


def _weight_key(*arrs):
    import hashlib
    h = hashlib.sha1()
    for a in arrs:
        h.update(np.ascontiguousarray(a, np.float32).tobytes())
    return h.hexdigest()


def kernel(x, w_qkv, b_qkv, w_proj, b_proj):
    import jax
    fn, in_names, out_names, out_avals, mesh = _get_runner()
    import ml_dtypes
    x = np.asarray(x, dtype=np.float32)
    xt = np.ascontiguousarray(
        np.transpose(x, (0, 2, 1))).astype(ml_dtypes.bfloat16)   # [B, D, T]
    xt_flat = xt.reshape(N_CORES * D, T)

    wk = _weight_key(w_qkv, b_qkv, w_proj, b_proj)
    if _CACHE.get("wkey") != wk:
        import ml_dtypes
        wqt = np.ascontiguousarray(
            np.asarray(w_qkv, np.float32).T).astype(ml_dtypes.bfloat16)
        wpt = np.ascontiguousarray(
            np.asarray(w_proj, np.float32).T).astype(ml_dtypes.bfloat16)
        bv = np.asarray(b_qkv, np.float32)[2 * D:]
        bp_eff = (np.asarray(b_proj, np.float32)
                  + bv @ np.asarray(w_proj, np.float32).T)
        host_w = {
            "wT_qkv": wqt,
            "b_qkv": np.asarray(b_qkv, np.float32),
            "wT_proj": wpt,
            "b_proj": bp_eff,
        }
        _CACHE["wdev"] = {k: jax.device_put(v) for k, v in host_w.items()}
        _CACHE["wkey"] = wk
    wdev = _CACHE["wdev"]

    args = []
    for n in in_names:
        args.append(xt_flat if n == "xT" else wdev[n])
    for a in out_avals:
        args.append(np.zeros((N_CORES * a.shape[0], *a.shape[1:]), a.dtype))
    outs = fn(*args)
    jax.block_until_ready(outs)
    oi = out_names.index("out")
    return np.asarray(outs[oi]).reshape(N_CORES, T, D).astype(np.float32)

